# revision 9
# baseline (speedup 1.0000x reference)
"""DirectNormLoss kernel for Trainium2 (Bass/Tile), 8-core data-parallel.

loss = (1/B) * sum_b [ 1 - <s_b, c_{l_b}> / (||c_{l_b}|| * max(||s_b||, ||t_b||)) ]

Strategy (v4, PE-centric with DoubleRow fp8, no gather):
  Host bin-packs the 1000 classes into 8 bins of exactly 2048 samples and
  <=128 distinct classes.  Each core receives:
    - its samples' s rows TRANSPOSED and pair-interleaved (d = 256c+2p+i)
      in fp8e4 for DoubleRow matmuls (K=256 per pass),
    - its samples' t rows row-major in fp8e4,
    - a 128-row local class table E (host L2-normalized, x64), transposed
      and pair-interleaved the same way,
    - a one-hot matrix OH[c_loc, b] selecting each sample's class.
  Per 512-sample phase (all DoubleRow, 8 K-chunks):
    - PE: dots_all[c, b] = <e_c, s_b> via 8 chunk matmuls into PSUM
    - PE: Gram blocks diag(S_blk^T S_blk) -> ||s_b||^2 (no vector-engine
      square pass at all)
    - DVE: identity-mask Gram -> s2 columns; OH-mask dots
    - PE: per-block ones-matmul -> dsel[128, 16]
  t-norms stream on the gpsimd SWDGE queue, squared on ACT (a few on DVE).
  Final: contrib = dsel / (64*sqrt(max(s2,t2))); host computes
  loss = (B - sum(partials)) / B.
"""

import numpy as np

import concourse.bass as bass  # noqa: F401
from concourse import bacc, mybir
from concourse.bass_utils import run_bass_kernel_spmd

# Problem constants (hardcoded per contract).
B_FULL = 16384
D = 2048
NUM_CLASS = 1000
N_CORES = 8
B_CORE = B_FULL // N_CORES          # 2048
P = 128
N_DR = D // (2 * P)                 # 8 DoubleRow K-chunks (256 d each)
N_TILES = B_CORE // P               # 16 sample tiles
N_PHASES = 4
TPP = N_TILES // N_PHASES           # 4 sample blocks per phase
NP = TPP * P                        # 512 samples per phase
N_ACT_T = 10                        # t^2 tiles on ACT (rest on DVE)
E_SCALE = 64.0
ND_WEIGHT = 1.0

_PROG = None


def _build():
    import concourse.tile as tile

    nc = bacc.Bacc("TRN2", target_bir_lowering=False, debug=False,
                   num_devices=N_CORES)

    F8 = mybir.dt.float8e4
    BF = mybir.dt.bfloat16
    FT = mybir.dt.float32
    Alu = mybir.AluOpType
    Act = mybir.ActivationFunctionType
    DR = mybir.MatmulPerfMode.DoubleRow

    # s_t[ph, c, p, i*NP+b] = s[sample NP*ph+b, d=256c+2p+i]
    s_ap = nc.dram_tensor("s_t", [N_PHASES, N_DR, P, 2 * NP], F8,
                          kind="ExternalInput").ap()
    t_ap = nc.dram_tensor("t_t", [N_TILES, P, D], F8,
                          kind="ExternalInput").ap()
    # et[p, (c,i,j)] = E_loc[class j, d=256c+2p+i]
    e_ap = nc.dram_tensor("et", [P, D], F8, kind="ExternalInput").ap()
    oh_ap = nc.dram_tensor("oh", [P, B_CORE], F8, kind="ExternalInput").ap()
    it_ap = nc.dram_tensor("it", [P, TPP, P], F8, kind="ExternalInput").ap()
    out_ap = nc.dram_tensor("out", [1, 1], FT, kind="ExternalOutput").ap()

    with tile.TileContext(nc) as tc:
        with (
            tc.tile_pool(name="sio", bufs=6) as sio,
            tc.tile_pool(name="tio", bufs=6) as tio,
            tc.tile_pool(name="dump", bufs=4) as dump,
            tc.tile_pool(name="msk", bufs=2) as msk,
            tc.tile_pool(name="stats", bufs=8) as stats,
            tc.tile_pool(name="persist", bufs=1) as persist,
            tc.tile_pool(name="psum", bufs=2, space="PSUM") as psum_pool,
            tc.tile_pool(name="psum1", bufs=1, space="PSUM") as psum1,
        ):
            # E table first on the sync queue: first dots matmul needs it.
            et_sb = persist.tile([P, N_DR, 2, P], F8)
            nc.sync.dma_start(
                out=et_sb[:].rearrange("p c i j -> p (c i j)"), in_=e_ap)
            # OH/identity needed only at phase tails; gpsimd SWDGE queue.
            oh_sb = persist.tile([P, B_CORE], F8)
            nc.gpsimd.dma_start(out=oh_sb[:], in_=oh_ap)
            it_sb = persist.tile([P, TPP, P], F8)
            nc.gpsimd.dma_start(out=it_sb[:], in_=it_ap)

            ones_bf = persist.tile([P, 1], BF)
            nc.vector.memset(ones_bf[:], 1.0)
            onesf = persist.tile([P, 1], FT)
            nc.vector.memset(onesf[:], 1.0)

            # Dummy Sqrt pulls the sqrt_and_friends act-table load
            # (covers Square/Copy/Sqrt) off the critical path.
            dum = persist.tile([1, 1], FT)
            nc.vector.memset(dum[:], 1.0)
            dumo = persist.tile([1, 1], FT)
            nc.scalar.activation(out=dumo[:], in_=dum[:], func=Act.Sqrt)

            s2a = persist.tile([P, N_TILES], FT)
            t2a = persist.tile([P, N_TILES], FT)
            dsel_ps = psum1.tile([P, N_TILES], FT)

            acc_all = persist.tile([P, N_TILES], FT)
            for ph in range(N_PHASES):
                lo = NP * ph
                dots_ps = psum_pool.tile([P, NP], FT, tag="dots")
                s2_ps = psum_pool.tile([P, TPP, P], FT, tag="s2")

                for g in range(4):
                    # t-pair DMA first so squares start early; alternate
                    # the two HWDGE queues between t and s traffic.
                    if g % 2 == 0:
                        t = 4 * ph + g
                        t_g = tio.tile([P, 2, D], F8, tag="t")
                        teng = nc.scalar if (ph + g) % 2 == 0 else nc.sync
                        teng.dma_start(
                            out=t_g[:],
                            in_=t_ap[t:t + 2].rearrange("k p d -> p k d"))
                        for u in range(2):
                            tv = t_g[:, u, :]
                            d0 = dump.tile([P, D], BF, tag="dump")
                            if (t + u) % 8 < 5:
                                nc.scalar.activation(
                                    out=d0[:], in_=tv, func=Act.Square,
                                    accum_out=t2a[:, t + u:t + u + 1])
                            else:
                                nc.vector.scalar_tensor_tensor(
                                    out=d0[:], in0=tv, scalar=1.0, in1=tv,
                                    op0=Alu.mult, op1=Alu.mult,
                                    accum_out=t2a[:, t + u:t + u + 1])
                    # two DoubleRow K-chunks per DMA group
                    s_g = sio.tile([P, 2, 2, NP], F8, tag="s")
                    eng = nc.sync if g % 2 == 0 else nc.scalar
                    eng.dma_start(
                        out=s_g[:].rearrange("p c i b -> p c (i b)"),
                        in_=s_ap[ph, 2 * g:2 * g + 2]
                        .rearrange("c p x -> p c x"))
                    for j in range(2):
                        c = 2 * g + j
                        rhs = s_g[:, j, :, :]           # [P, 2, NP]
                        nc.tensor.matmul(
                            out=dots_ps[:], lhsT=et_sb[:, c, :, :],
                            rhs=rhs, perf_mode=DR,
                            start=(c == 0), stop=(c == N_DR - 1))
                        for blk in range(TPP):
                            bs = rhs[:, :, P * blk:P * (blk + 1)]
                            nc.tensor.matmul(
                                out=s2_ps[:, blk, :], lhsT=bs, rhs=bs,
                                perf_mode=DR,
                                start=(c == 0), stop=(c == N_DR - 1))

                # phase tail: extract s2 diag, mask dots, reduce to dsel
                msk2 = msk.tile([P, TPP, P], BF, tag="m2")
                nc.vector.scalar_tensor_tensor(
                    out=msk2[:], in0=s2_ps[:], scalar=1.0, in1=it_sb[:],
                    op0=Alu.mult, op1=Alu.mult)
                nc.vector.tensor_reduce(
                    out=s2a[:, TPP * ph:TPP * (ph + 1)], in_=msk2[:],
                    axis=mybir.AxisListType.X, op=Alu.add)
                mskd = msk.tile([P, NP], BF, tag="md")
                nc.vector.scalar_tensor_tensor(
                    out=mskd[:], in0=dots_ps[:], scalar=1.0,
                    in1=oh_sb[:, lo:lo + NP], op0=Alu.mult, op1=Alu.mult)
                for blk in range(TPP):
                    col = TPP * ph + blk
                    nc.tensor.matmul(
                        out=dsel_ps[:, col:col + 1],
                        lhsT=mskd[:, P * blk:P * (blk + 1)],
                        rhs=ones_bf[:], start=True, stop=True)

                # per-phase mini-chain: only the last phase's sits in the
                # tail.  contrib = dsel / (64*sqrt(max(s2,t2)))
                sl = slice(TPP * ph, TPP * (ph + 1))
                m2 = stats.tile([P, TPP], FT, tag="m2")
                nc.vector.tensor_tensor(out=m2[:], in0=s2a[:, sl],
                                        in1=t2a[:, sl], op=Alu.max)
                rn = stats.tile([P, TPP], FT, tag="rn")
                nc.scalar.activation(out=rn[:], in_=m2[:], func=Act.Sqrt,
                                     scale=float(E_SCALE * E_SCALE))
                rs = stats.tile([P, TPP], FT, tag="rs")
                nc.vector.reciprocal(out=rs[:], in_=rn[:])
                nc.vector.tensor_tensor(out=acc_all[:, sl],
                                        in0=dsel_ps[:, sl], in1=rs[:],
                                        op=Alu.mult)

            rsum = stats.tile([P, 1], FT, tag="rsum")
            nc.vector.tensor_reduce(out=rsum[:], in_=acc_all[:],
                                    axis=mybir.AxisListType.X, op=Alu.add)
            total = psum1.tile([1, 1], FT)
            nc.tensor.matmul(out=total[:], lhsT=rsum[:], rhs=onesf[:],
                             start=True, stop=True)
            res = persist.tile([1, 1], FT)
            nc.scalar.activation(out=res[:], in_=total[:], func=Act.Copy)
            nc.sync.dma_start(out=out_ap[:], in_=res[:])

    nc.compile()
    return nc


def _get_program():
    global _PROG
    if _PROG is None:
        _PROG = _build()
    return _PROG


def _pack_bins(labels):
    """Assign classes to 8 bins: exactly B_CORE samples, <=128 classes."""
    counts = np.bincount(labels, minlength=NUM_CLASS)
    loads = np.zeros(N_CORES, dtype=np.int64)
    bins = [[] for _ in range(N_CORES)]          # (class, take, off)
    leftovers = []
    for c in np.argsort(counts)[::-1]:
        sz = int(counts[c])
        if sz == 0:
            continue
        cand = [i for i in range(N_CORES)
                if loads[i] + sz <= B_CORE and len(bins[i]) < 126]
        if cand:
            i = min(cand, key=lambda i: loads[i])
            bins[i].append((int(c), sz, 0))
            loads[i] += sz
        else:
            leftovers.append((int(c), sz))
    for c, sz in leftovers:
        off = 0
        for i in np.argsort(loads):
            if off >= sz:
                break
            cap = int(B_CORE - loads[i])
            if cap <= 0:
                continue
            take = min(cap, sz - off)
            bins[i].append((c, take, int(off)))
            loads[i] += take
            off += take
        assert off == sz, "couldn't place split class"
    assert all(l == B_CORE for l in loads)
    assert all(len(b) <= P for b in bins)
    return bins


def _make_in_maps(s_emb, t_emb, T_EMB, labels):
    import ml_dtypes
    FP8 = ml_dtypes.float8_e4m3

    s_emb = np.asarray(s_emb, dtype=np.float32)
    t_emb = np.asarray(t_emb, dtype=np.float32)
    T_EMB = np.asarray(T_EMB, dtype=np.float32)
    labels = np.asarray(labels).astype(np.int64)

    bins = _pack_bins(labels)
    order = np.argsort(labels, kind="stable")
    starts = np.zeros(NUM_CLASS + 1, dtype=np.int64)
    np.cumsum(np.bincount(labels, minlength=NUM_CLASS), out=starts[1:])

    # identity tiled TPP times (shared across cores)
    it = np.zeros((P, TPP, P), dtype=FP8)
    idx = np.arange(P)
    for k in range(TPP):
        it[idx, k, idx] = 1.0

    in_maps = []
    for i in range(N_CORES):
        cls = [c for c, _, _ in bins[i]]
        sel = np.concatenate([
            order[starts[c] + off:starts[c] + off + take]
            for c, take, off in bins[i]])
        assert sel.shape[0] == B_CORE
        lab_loc = np.concatenate([
            np.full(take, j, dtype=np.int64)
            for j, (_, take, _) in enumerate(bins[i])])

        S = s_emb[sel]                               # [B_CORE, D]
        # [ph, c, p, i*NP+b] with d = 256c+2p+i, sample = NP*ph+b
        s_t = np.ascontiguousarray(
            S.T.reshape(N_DR, P, 2, N_PHASES, NP)
            .transpose(3, 0, 1, 2, 4)
            .reshape(N_PHASES, N_DR, P, 2 * NP)).astype(FP8)
        t_t = np.ascontiguousarray(
            t_emb[sel].reshape(N_TILES, P, D)).astype(FP8)

        E = np.zeros((P, D), dtype=np.float32)
        rows = T_EMB[cls]
        E[:len(cls)] = rows / np.linalg.norm(rows, axis=1, keepdims=True)
        E *= E_SCALE
        # et[p, c, i, j] = E[j, 256c+2p+i]
        et = np.ascontiguousarray(
            E.T.reshape(N_DR, P, 2, P).transpose(1, 0, 2, 3)).astype(FP8)

        oh = np.zeros((P, B_CORE), dtype=FP8)
        oh[lab_loc, np.arange(B_CORE)] = 1.0

        in_maps.append({
            "s_t": s_t,
            "t_t": t_t,
            "et": et,
            "oh": oh,
            "it": it,
        })
    return in_maps


def run(s_emb, t_emb, T_EMB, labels, trace=False, **spmd_kwargs):
    """Run on 8 NeuronCores; returns (loss_scalar, BassKernelResults)."""
    nc = _get_program()
    in_maps = _make_in_maps(s_emb, t_emb, T_EMB, labels)
    res = run_bass_kernel_spmd(nc, in_maps, core_ids=list(range(N_CORES)),
                               trace=trace, **spmd_kwargs)
    partials = [res.results[i]["out"][0, 0] for i in range(N_CORES)]
    total = np.sum(np.asarray(partials, dtype=np.float64))
    loss = np.array((B_FULL - total) * ND_WEIGHT / B_FULL, dtype=np.float32)
    return loss, res


def kernel(s_emb, t_emb, T_EMB, labels):
    loss, _ = run(s_emb, t_emb, T_EMB, labels)
    return loss
